# revision 48
# baseline (speedup 1.0000x reference)
"""SoftDTW loss (AbstractDTW, gamma=1) Trainium2 Bass kernel — v2.

Algorithm: exp-space linearization of the SoftDTW DP. With E = exp(-R) and
W = exp(-D), the recurrence R[i,j] = D[i,j] + softmin(R[i-1,j-1], R[i-1,j],
R[i,j-1]) becomes linear:

    E[i,j] = W[i,j] * (E[i-1,j-1] + E[i-1,j] + E[i,j-1])

v2 core trick: the per-row pair-add (E[i-1,j-1] + E[i-1,j]) is FUSED into a
single DVE tensor_tensor_scan of 2C interleaved elements, using overlapping
multi-free-dim access patterns (emitted as a raw InstTensorScalarPtr, since
the bass wrapper only accepts 2-D operands).  Element stream per cell j:

    m=(j,0): state = (eprev[2j]   + state) * ones[j]     # pair partner 1
    m=(j,1): state = (eprev[2j+2] + state) * w[j]        # partner 2, * W

so one scan instruction advances a whole row — the separate [P,C] pair-add
(and its ~220ns of RAW-semaphore latency) disappears from the critical chain.
Per step the DVE runs exactly one 128-element scan: ~300ns/step vs ~450ns.

e-tiles are [P, 2C+1]: pos 0 is a permanent 0 (the boundary enters via the
scan's initial operand instead), E_j at pos 2+2j, garbage at odd positions.
The slot-boundary value is accumulated on the PE into PSUM (two shift-matmuls
per step: last cols of rows i-1 and i of slot k-1), and the scan reads it
directly as its PSUM initial operand — scalar operands are free of the PSUM
access penalty, so no boundary-import instruction touches the chain.  The
power-of-two boundary scale ratio r is folded into the W tile's (j=0, r=0)
multiplier slot (the "ones" block), written once per rescale epoch on DVE
(Pool's in-order queue sits behind 2.1us W-ops and would stall the epoch
scans); at epoch steps {T, T+1} the slot carries r_old and the sf part is
pre-multiplied into the PSUM pair, matching the baseline's staged
import*r-then-*sf fixups (a single combined factor under/overflows f32
whenever adjacent-slot scales differ by >2^126).

W production: dx/dy subs on Pool (tensor_tensor; TensorScalarPtr is not in
Pool's ISA), Square/Exp on Act, and d=sqx+sqy on DVE as per-row [P,1,C]
slices draining one per step (drain_dq) so they mostly hide in the scan
chain's ~95ns RAW gaps.  W chunks are [P, RBLK, 2C]: first C columns per
step are the "ones" block (memset once; epoch r-values overwrite slot 0),
w row in the last C.

Numerical range: per-slot power-of-2 scales re-anchored every Q=16 steps
from an anchor sampled DLEAD=8 steps early.  The anchor reduces over the
row's even positions plus the pos-1 boundary product only — including the
odd-position garbage runs a few bits high and pushes deep-tail boundary
exports below the f32 denormal floor.  HEADROOM -199 (anchor peak 2^72 vs
v1's 2^64) adds 8 bits of deep-tail range for the same reason.  The final
CxC block is recomputed on host in float64 log space from exported boundary
values.  Batch dim (64) sharded 8 ways across cores; mean on host.
"""

import sys
from contextlib import ExitStack

import numpy as np

sys.path.insert(0, "/opt/trn_rl_repo")

import bass_rust  # noqa: E402
import concourse.bass as bass  # noqa: E402
import concourse.tile as tile  # noqa: E402
from concourse import bacc, mybir  # noqa: E402
from concourse import bass_utils  # noqa: E402

AF = mybir.ActivationFunctionType
ALU = mybir.AluOpType

NCORES = 8
B = 8          # batch per core
K = 16         # column slots
N = 1024       # sequence length
C = N // K     # columns per slot
P = K * B      # 128 partitions
EW = 2 * C + 1                   # e-tile width (pos0 + interleaved row)
NSTEP = N + 2 * (K - 1)          # 1054 wavefront steps
RBLK = 64                        # W-production chunk (steps)
QTR = 16                         # production slice (steps)
NCHUNK = (NSTEP + RBLK - 1) // RBLK
SXLEN = NCHUNK * RBLK            # staggered snake buffer length (1088)
Q = 16                           # rescale cadence
DLEAD = 8                        # rescale anchor staleness (steps)
RESCALE_STEPS = [t for t in range(Q, NSTEP) if t % Q == 0]
PAD = np.float32(1e4)            # out-of-range snake pad -> W == 0
TAIL0 = ((N - C - 1 + 2 * (K - 2)) // Q) * Q   # export window start step
NTAIL = NSTEP - TAIL0                          # boundary-column exports
EROW_STEP = N - C - 1 + 2 * (K - 1)            # step producing row N-C-1
NHIST = (NSTEP - 1 - TAIL0) // Q + 1           # lacc snapshots in window
HEADROOM = -199.0                # anchor 2^72 peak: +8 bits of deep-tail
                                 # range for the boundary exports vs v1's
                                 # 2^64 (wake sf-clamp overshoot still heals
                                 # in one epoch)
# filler with exponent field == -HEADROOM so empty slots keep lcand == lacc
FILLER = np.float32(2.0 ** (-HEADROOM - 127.0))
EXP_ONE = 127 << 23              # f32 bit pattern of 1.0
DBG_STEPS = []                   # e-tile snapshot steps (debug builds only)


def build_bass():
    """Build the per-core Bass program (SPMD: same program on all cores)."""
    nc = bacc.Bacc(
        "TRN2",
        target_bir_lowering=False,
        debug=False,
        enable_asserts=False,
        num_devices=NCORES,
    )
    f32 = mybir.dt.float32
    cx_d = nc.dram_tensor("cx", [P, C], f32, kind="ExternalInput").ap()
    cy_d = nc.dram_tensor("cy", [P, C], f32, kind="ExternalInput").ap()
    sx_d = nc.dram_tensor("sx", [P, SXLEN], f32, kind="ExternalInput").ap()
    sy_d = nc.dram_tensor("sy", [P, SXLEN], f32, kind="ExternalInput").ap()
    sh_d = nc.dram_tensor("shift", [P, P], f32, kind="ExternalInput").ap()
    out_d = nc.dram_tensor("out", [P, 2], f32, kind="ExternalOutput").ap()
    bcol_d = nc.dram_tensor("bcol", [P, NTAIL], f32, kind="ExternalOutput").ap()
    erow_d = nc.dram_tensor("erow", [P, C + 1], f32, kind="ExternalOutput").ap()
    lh_d = nc.dram_tensor("lh", [P, NHIST], f32, kind="ExternalOutput").ap()
    dbg_d = None
    if DBG_STEPS:
        dbg_d = nc.dram_tensor(
            "dbg", [P, len(DBG_STEPS) * (EW + 1)], f32, kind="ExternalOutput"
        ).ap()

    with TileKernel(nc) as tk:
        tk.body(cx_d, cy_d, sx_d, sy_d, sh_d, out_d, bcol_d, erow_d, lh_d,
                dbg_d)
    nc.compile()
    return nc


class TileKernel:
    def __init__(self, nc):
        self.nc = nc
        self.ctx = ExitStack()
        self.tc = tile.TileContext(nc)

    def __enter__(self):
        self.ctx.__enter__()
        self.tc.__enter__()
        return self

    def __exit__(self, *a):
        self.ctx.__exit__(*a)  # close tile pools before scheduling
        return self.tc.__exit__(*a)

    # ------------------------------------------------------------ fused scan
    def fused_scan(self, eprev, ecur, wt, off, init):
        """One wavefront row: 2C-element interleaved scan.

        eprev/ecur: [P, EW] e-tiles.  wt: [P, RBLK, 2C] W chunk, step `off`.
        init: AP ([P,1], SBUF or PSUM) or float immediate."""
        nc = self.nc
        # data0: (j,r) -> eprev pos 2j + 2r   (even positions: pos0 + E's)
        d0 = eprev[:, 0:C].unsqueeze(2).broadcast_to([P, C, 2]).copy()
        d0.ap = bass_rust.VecI64Pair([[EW, P], [2, C], [2, 2]])
        # data1: (j,r) -> wt[off] pos j + r*C  ([ones-block | w-row])
        d1 = wt[:, off, 0:C].unsqueeze(2).broadcast_to([P, C, 2]).copy()
        d1.ap = bass_rust.VecI64Pair([[RBLK * 2 * C, P], [1, C], [C, 2]])
        # out: (j,r) -> ecur pos 1 + 2j + r  (odd garbage, even E_j)
        o = ecur[:, 1:1 + C].unsqueeze(2).broadcast_to([P, C, 2]).copy()
        o.ap = bass_rust.VecI64Pair([[EW, P], [2, C], [1, 2]])
        if isinstance(init, float):
            init_arg = nc.vector.lower_ap_or_imm(init)
        else:
            init_arg = nc.vector.lower_ap(init)
        nc.vector.add_instruction(
            mybir.InstTensorScalarPtr(
                name=nc.get_next_instruction_name(),
                is_tensor_tensor_scan=True,
                is_scalar_tensor_tensor=True,
                op0=ALU.add,
                op1=ALU.mult,
                ins=[nc.vector.lower_ap(d0), init_arg, nc.vector.lower_ap(d1)],
                outs=[nc.vector.lower_ap(o)],
            )
        )

    def body(self, cx_d, cy_d, sx_d, sy_d, sh_d, out_d, bcol_d, erow_d, lh_d,
             dbg_d=None):
        nc = self.nc
        tc = self.tc
        ctx = self.ctx
        f32 = mybir.dt.float32
        u32 = mybir.dt.uint32

        const = ctx.enter_context(tc.tile_pool(name="const", bufs=1))
        qpool = ctx.enter_context(tc.tile_pool(name="qp", bufs=2))
        small = ctx.enter_context(tc.tile_pool(name="sm", bufs=2))
        psum = ctx.enter_context(tc.tile_pool(name="ps", bufs=3, space="PSUM"))
        psum2 = ctx.enter_context(
            tc.tile_pool(name="ps2", bufs=1, space="PSUM")
        )
        self.psum2 = psum2

        # ---- constants / inputs
        cx = const.tile([P, C], f32)
        cy = const.tile([P, C], f32)
        sx = const.tile([P, SXLEN], f32)
        sy = const.tile([P, SXLEN], f32)
        shm = const.tile([P, P], f32)
        nc.sync.dma_start(cx[:], cx_d)
        nc.sync.dma_start(cy[:], cy_d)
        nc.sync.dma_start(sx[:], sx_d)
        nc.sync.dma_start(sy[:], sy_d)
        nc.sync.dma_start(shm[:], sh_d)

        # accumulated base-2 exponent per partition (exact f32 integers)
        lacc_a = const.tile([P, 1], f32)
        lacc_b = const.tile([P, 1], f32)
        nc.vector.memset(lacc_a[:], 0.0)
        self.lacc_cur, self.lacc_alt = lacc_a, lacc_b
        # r per scale regime: 2^(L[p-8]-L[p]) as bitcast-f32 (power of 2)
        r_a = const.tile([P, 1], u32)
        r_b = const.tile([P, 1], u32)
        nc.vector.memset(r_a[:].bitcast(f32), 1.0)
        self.r_tiles = [r_a, r_b]

        # per-epoch value scale factor 2^(Lacc-Lnew) bits
        self.sf_t = const.tile([P, 1], u32)
        # additive mask: slot 0 rows get a huge negative so max() keeps own
        negmask = const.tile([P, 1], f32)
        nc.vector.memset(negmask[:], 0.0)
        nc.vector.memset(negmask[0:B, :], -3.0e8)
        # DP seed: scan_0 initial = 1 on slot-0 partitions
        seed = const.tile([P, 1], f32)
        nc.vector.memset(seed[:], 0.0)
        nc.vector.memset(seed[0:B, :], 1.0)

        bcol = const.tile([P, NTAIL], f32)
        erow = const.tile([P, C + 1], f32)
        lh = const.tile([P, NHIST], f32)
        dbg = None
        if DBG_STEPS:
            dbg = const.tile([P, len(DBG_STEPS) * (EW + 1)], f32)

        # W chunk ring: explicit double buffer, [ones-block | w-row] per step
        wbufs = [const.tile([P, RBLK, 2 * C], f32, name=f"wb{i}")
                 for i in range(2)]
        # e-tile ring: explicit, pos0 kept at zero permanently
        ebufs = [const.tile([P, EW], f32, name=f"eb{i}") for i in range(7)]

        self.nc_ = nc
        self.cx, self.cy, self.sx, self.sy = cx, cy, sx, sy
        self.shm, self.negmask = shm, negmask
        self.qpool, self.psum, self.small = qpool, psum, small
        self.lh, self.f32, self.u32 = lh, f32, u32
        self.wbufs = wbufs

        # ones blocks: the data1 even-element multipliers (w-rows are
        # written by Exp; epoch r-values overwrite slot 0 per step)
        nc.gpsimd.memset(wbufs[0][:, :, 0:C], 1.0)
        nc.gpsimd.memset(wbufs[1][:, :, 0:C], 1.0)
        # e ring: zero everything once (pos0 must stay 0)
        for eb in ebufs:
            nc.vector.memset(eb[:], 0.0)

        # ---- prologue: produce W chunk 0 quarters 0-1 on DVE+Act so the
        # chain starts early; quarters 2-3 go to Pool at loop start
        w_tiles = {0: wbufs[0]}
        for q in (0, 1):
            for s in self.produce_quarter(0, q, wbufs[0], nc.vector)[0:6]:
                s()

        e_tiles = {-1: ebufs[6]}

        self.rst = {}  # live rescale-chain state
        self.dq_sched = {}  # step -> [dq slice lambdas, exp lambda]
        self.dq_fifo = []
        ps_tiles = {}  # step -> psum tile with accumulated boundary pair

        for t in range(NSTEP):
            cchunk = t // RBLK
            off = t % RBLK
            if t == 0:
                for q in (2, 3):
                    for s in self.produce_quarter(0, q, wbufs[0], nc.gpsimd)[0:6]:
                        s()
            # ---- W production for chunk c+1 on Pool/Act + per-step dq
            if cchunk + 1 < NCHUNK:
                self.produce_slice(cchunk + 1, off, w_tiles)
            self.drain_dq(t)

            # ---- rescale chain (tiny ops in chain gaps + PE routes)
            Tnext = ((t + DLEAD) // Q) * Q
            if Tnext in RESCALE_STEPS and t >= Tnext - DLEAD:
                self.rescale_phase(t - (Tnext - DLEAD), Tnext, e_tiles)

            # ---- wavefront step t
            is_epoch = t in RESCALE_STEPS
            eprev = e_tiles[t - 1]
            ecur = ebufs[t % 6]
            e_tiles[t] = ecur

            if is_epoch:
                sf_ap = self.sf_t[:].bitcast(f32)
                nc.vector.tensor_scalar_mul(eprev[:], eprev[:], sf_ap)

            if t == 0:
                init = seed[:]
            elif t == 1:
                init = 0.0
            else:
                init = ps_tiles[t][:]
            self.fused_scan(eprev, ecur, w_tiles[cchunk], off, init)

            if DBG_STEPS and t in DBG_STEPS:
                di = DBG_STEPS.index(t) * (EW + 1)
                nc.vector.tensor_copy(dbg[:, di:di + EW], ecur[:])
                nc.vector.tensor_copy(
                    dbg[:, di + EW:di + EW + 1], self.lacc_cur[:]
                )
            if t >= TAIL0:
                nc.vector.tensor_copy(
                    bcol[:, t - TAIL0:t - TAIL0 + 1],
                    ecur[:, 2 * C:2 * C + 1],
                )
            if t == EROW_STEP:
                # boundary col: single-routed E[i0-1, j0-1] * r
                regime = (t - 2) // Q
                r_ap = self.r_tiles[regime % 2][:].bitcast(f32)
                nc.vector.tensor_scalar_mul(
                    erow[:, 0:1], self.ps_er[:], r_ap
                )
                # strided row: E_j at even positions 2..2C
                src = ecur[:, 2:2 + C].copy()
                src.ap = bass_rust.VecI64Pair([[EW, P], [2, C]])
                nc.vector.tensor_copy(erow[:, 1:C + 1], src)
                nc.sync.dma_start(erow_d, erow[:])
            if t == 1041:
                # lh fully written by the T=1040 epoch; overlap its export
                nc.sync.dma_start(lh_d, lh[:])
                nc.sync.dma_start(bcol_d[:, 0:64], bcol[:, 0:64])

            # ---- boundary routes: accumulate last cols of rows t-1, t of
            # slot k-1 (shift partitions +8) into PSUM for scan at t+2
            if t + 2 < NSTEP:
                ps = psum.tile([P, 1], f32, tag="sh", name=f"ps{t + 2}")
                nc.tensor.matmul(
                    ps[:], shm[:], eprev[:, 2 * C:2 * C + 1],
                    start=True, stop=False,
                )
                nc.tensor.matmul(
                    ps[:], shm[:], ecur[:, 2 * C:2 * C + 1],
                    start=False, stop=True,
                )
                # epoch steps {T, T+1} consume their boundary in the NEW
                # scale: pre-multiply the PSUM pair by sf (the data1 slot
                # only carries r_old)
                if (t + 2) % Q in (0, 1) and ((t + 2) // Q) * Q in RESCALE_STEPS:
                    nc.vector.tensor_scalar_mul(
                        ps[:], ps[:], self.sf_t[:].bitcast(f32)
                    )
                ps_tiles[t + 2] = ps
            if t == EROW_STEP - 2:
                # single route (not accumulated) for the erow boundary
                self.ps_er = self.psum2.tile([P, 1], f32, tag="er")
                nc.tensor.matmul(
                    self.ps_er[:], shm[:], ecur[:, 2 * C:2 * C + 1],
                    start=True, stop=True,
                )
            ps_tiles.pop(t, None)
            e_tiles.pop(t - 5, None)

        # ---- finalization: out = [E_last, Lacc]
        outt = const.tile([P, 2], f32)
        nc.vector.tensor_copy(
            outt[:, 0:1], e_tiles[NSTEP - 1][:, 2 * C:2 * C + 1]
        )
        nc.vector.tensor_copy(outt[:, 1:2], self.lacc_cur[:])
        nc.sync.dma_start(out_d, outt[:])
        nc.sync.dma_start(bcol_d[:, 64:NTAIL], bcol[:, 64:NTAIL])
        if DBG_STEPS:
            nc.sync.dma_start(dbg_d, dbg[:])

    # ------------------------------------------------------------------ W
    def produce_quarter(self, chunk, q, wt, veng):
        """Produce w rows [C:2C] of steps q*QTR..(q+1)*QTR of `chunk`.
        veng: tensor-op engine namespace (nc.vector or nc.gpsimd)."""
        nc = self.nc_
        f32 = self.f32
        t0 = chunk * RBLK + q * QTR
        nm = f"{chunk}_{q}"
        dxq = self.qpool.tile([P, QTR, C], f32, tag="dx", name="dx" + nm)
        dyq = self.qpool.tile([P, QTR, C], f32, tag="dy", name="dy" + nm)
        sqx = self.qpool.tile([P, QTR, C], f32, tag="sqx", name="sqx" + nm)
        sqy = self.qpool.tile([P, QTR, C], f32, tag="sqy", name="sqy" + nm)
        dq = self.qpool.tile([P, QTR, C], f32, tag="dq", name="dq" + nm)
        cxb = self.cx[:].unsqueeze(1).broadcast_to([P, QTR, C])
        cyb = self.cy[:].unsqueeze(1).broadcast_to([P, QTR, C])
        sxb = self.sx[:, t0:t0 + QTR].unsqueeze(2).broadcast_to([P, QTR, C])
        syb = self.sy[:, t0:t0 + QTR].unsqueeze(2).broadcast_to([P, QTR, C])
        wsl = wt[:, q * QTR:(q + 1) * QTR, C:2 * C]
        # the d = sqx + sqy pass always runs on DVE: Pool (2 subs) is the
        # busiest engine; per-row [P,1,C] slices mostly hide in chain gaps
        steps = [
            lambda: veng.tensor_sub(dxq[:], cxb, sxb),
            lambda: nc.scalar.activation(sqx[:], dxq[:], AF.Square),
            lambda: veng.tensor_sub(dyq[:], cyb, syb),
            lambda: nc.scalar.activation(sqy[:], dyq[:], AF.Square),
            lambda: nc.vector.tensor_add(dq[:], sqx[:], sqy[:]),
            lambda: nc.scalar.activation(wsl, dq[:], AF.Exp, scale=-1.0),
        ]
        steps.extend(
            (lambda i=i: nc.vector.tensor_add(
                dq[:, i:i + 1], sqx[:, i:i + 1], sqy[:, i:i + 1]
            ))
            for i in range(QTR)
        )
        return steps

    def produce_slice(self, chunk, off, w_tiles):
        """Trigger quarter production: subs+squares (Pool/Act) at ph 1; the
        16 per-row dq slices drain one-per-step on DVE starting 16 steps
        later (driven by drain_dq), with the Exp right after the last."""
        q, ph = off // QTR, off % QTR
        if ph != 1:
            return
        if chunk * RBLK + q * QTR >= NSTEP:
            return  # w rows past the last wavefront step are never read
        if chunk not in w_tiles:
            w_tiles[chunk] = self.wbufs[chunk % 2]
        steps = self.produce_quarter(chunk, q, w_tiles[chunk], self.nc_.gpsimd)
        for s in steps[0:4]:   # dx, sqx, dy, sqy
            s()
        t_now = (chunk - 1) * RBLK + off
        self.dq_sched[t_now + QTR] = [steps[6:22], steps[5]]  # slices, exp

    def drain_dq(self, t):
        """One dq slice per step on DVE; the quarter's Exp is emitted with
        its 16th slice (Act, off-chain)."""
        ready = self.dq_sched.pop(t, None)
        if ready is not None:
            self.dq_fifo.append(ready)
        if self.dq_fifo:
            slices, exp = self.dq_fifo[0]
            slices.pop(0)()              # dq slice (DVE, [P,1,C])
            if not slices:
                exp()
                self.dq_fifo.pop(0)

    # ------------------------------------------------------------ rescale
    def w_slot_ap(self, t0, n):
        """AP of the data1 (j=0,r=0) multiplier slots for steps t0..t0+n-1.
        These live at position `off*2C + 0` of the W chunk buffer."""
        chunk = t0 // RBLK
        off = t0 % RBLK
        wt = self.wbufs[chunk % 2]
        return wt[:, off:off + n, 0:1]

    def rescale_phase(self, ph, T, e_tiles):
        """Tiny-op scale chain for epoch T, spread over groups T-8..T-1.
        Reads the row produced at step T-DLEAD-1 (stale anchor)."""
        nc = self.nc_
        f32, u32 = self.f32, self.u32
        st = self.rst.setdefault(("rs", T), {})
        sm = self.small
        X = mybir.AxisListType.X

        def tl(name, dt=f32):
            st[name] = sm.tile([P, 1], dt, tag="rs_" + name, name=f"rs_{name}_{T}")
            return st[name]

        if ph == 0:
            prev = e_tiles[T - DLEAD - 1]
            # anchor over the real row (even positions) + the boundary
            # product at pos 1 — excluding garbage keeps the scale choice
            # aligned with v1's (garbage prefix-sums run a few bits high,
            # which pushes deep-tail export cells below the denormal floor)
            evens = prev[:, 0:C + 1].copy()
            evens.ap = bass_rust.VecI64Pair([[EW, P], [2, C + 1]])
            nc.vector.tensor_reduce(
                tl("m")[:], evens, axis=X, op=ALU.max
            )
            nc.vector.tensor_tensor(
                st["m"][:], st["m"][:], prev[:, 1:2], op=ALU.max
            )
        elif ph == 1:
            nc.vector.tensor_scalar(
                tl("z")[:], st["m"][:], 1e-37, None, op0=ALU.is_le
            )
            nc.vector.scalar_tensor_tensor(
                tl("mz")[:], st["z"][:], float(FILLER), st["m"][:],
                op0=ALU.mult, op1=ALU.add,
            )
            nc.vector.tensor_scalar(
                tl("eu", u32)[:], st["mz"][:].bitcast(u32), 23, None,
                op0=ALU.logical_shift_right,
            )
        elif ph == 2:
            nc.vector.tensor_copy(tl("ef")[:], st["eu"][:])   # u32 -> f32
            nc.vector.scalar_tensor_tensor(
                tl("lc")[:], st["ef"][:], HEADROOM, self.lacc_cur[:],
                op0=ALU.add, op1=ALU.add,
            )
            nc.vector.scalar_tensor_tensor(
                tl("lsrc")[:], st["z"][:], -3.0e8, st["lc"][:],
                op0=ALU.mult, op1=ALU.add,
            )
        elif ph == 3:
            st["psl"] = self.psum2.tile([P, 1], f32, tag="psl", name=f"psl_{T}")
            nc.tensor.matmul(
                st["psl"][:], self.shm[:], st["lsrc"][:], start=True, stop=True
            )
            nc.vector.tensor_tensor(
                tl("nb")[:], st["psl"][:], self.negmask[:], op=ALU.add
            )
        elif ph == 4:
            nc.vector.tensor_tensor(
                tl("mx")[:], st["lsrc"][:], st["nb"][:], op=ALU.max
            )
            nc.vector.tensor_scalar(
                tl("vv")[:], st["nb"][:], -1e8, None, op0=ALU.is_ge
            )
            nc.vector.tensor_tensor(
                st["mx"][:], st["mx"][:], st["lc"][:], op=ALU.subtract
            )
        elif ph == 5:
            nc.vector.tensor_tensor(
                st["mx"][:], st["mx"][:], st["vv"][:], op=ALU.mult
            )
            lnew = self.lacc_alt
            st["lnew"] = lnew
            nc.vector.tensor_tensor(
                lnew[:], st["lc"][:], st["mx"][:], op=ALU.add
            )
            nc.vector.scalar_tensor_tensor(
                tl("sfe")[:], lnew[:], -1.0, self.lacc_cur[:],
                op0=ALU.mult, op1=ALU.add,
            )
        elif ph == 6:
            nc.vector.tensor_scalar(
                st["sfe"][:], st["sfe"][:], -126.0, 126.0,
                op0=ALU.max, op1=ALU.min,
            )
            nc.vector.tensor_scalar(
                st["sfe"][:], st["sfe"][:], 127.0, None, op0=ALU.add
            )
            nc.vector.tensor_copy(tl("sfu", u32)[:], st["sfe"][:])
            nc.vector.tensor_scalar(
                self.sf_t[:], st["sfu"][:], 23, None,
                op0=ALU.logical_shift_left,
            )
        elif ph == 7:
            # steps {T, T+1} boundary multiplier: r_old; the sf part of the
            # scale transition is applied to their PSUM values directly
            # (matching the baseline's staged import*r then *sf fixup).
            regime_old = (T // Q - 1) % 2
            sl = self.w_slot_ap(T, 2)
            nc.vector.tensor_scalar(
                sl, sl, 0.0, self.r_tiles[regime_old][:].bitcast(f32),
                op0=ALU.mult, op1=ALU.add,
            )
            st["psl2"] = self.psum2.tile([P, 1], f32, tag="psl2", name=f"psl2_{T}")
            nc.tensor.matmul(
                st["psl2"][:], self.shm[:], st["lnew"][:], start=True, stop=True
            )
            nc.vector.tensor_tensor(
                tl("dl")[:], st["psl2"][:], self.negmask[:], op=ALU.add
            )
            nc.vector.tensor_tensor(
                st["dl"][:], st["dl"][:], st["lnew"][:], op=ALU.subtract
            )

        if ph == DLEAD - 1:
            # finish r for the new regime; applied from step T+2 on
            nc.vector.tensor_scalar(
                st["dl"][:], st["dl"][:], -126.0, 110.0,
                op0=ALU.max, op1=ALU.min,
            )
            nc.vector.tensor_scalar(
                st["dl"][:], st["dl"][:], 127.0, None, op0=ALU.add
            )
            nc.vector.tensor_copy(tl("ru", u32)[:], st["dl"][:])
            regime = T // Q
            r_new = self.r_tiles[regime % 2]
            nc.vector.tensor_scalar(
                r_new[:], st["ru"][:], 23, None,
                op0=ALU.logical_shift_left,
            )
            # write r_new into multiplier slots for steps T+2 .. T+15
            if T + 2 < NSTEP:
                nn = min(Q - 2, NSTEP - (T + 2))
                sl = self.w_slot_ap(T + 2, nn)
                nc.vector.tensor_scalar(
                    sl, sl, 0.0, r_new[:].bitcast(f32),
                    op0=ALU.mult, op1=ALU.add,
                )
            if T >= TAIL0:
                kk = (T - TAIL0) // Q
                nc.vector.tensor_copy(
                    self.lh[:, kk:kk + 1], st["lnew"][:]
                )
            self.lacc_cur, self.lacc_alt = st["lnew"], self.lacc_cur
            del self.rst[("rs", T)]


def prep_core_inputs(snake, contour):
    """snake, contour: [B, N, 2] float32 -> input dict for one core."""
    cx = np.empty((P, C), np.float32)
    cy = np.empty((P, C), np.float32)
    sx = np.full((P, SXLEN), PAD, np.float32)
    sy = np.full((P, SXLEN), PAD, np.float32)
    for k in range(K):
        for b in range(B):
            p = k * B + b
            cx[p] = contour[b, k * C:(k + 1) * C, 0]
            cy[p] = contour[b, k * C:(k + 1) * C, 1]
            lo = 2 * k
            sx[p, lo:lo + N] = snake[b, :, 0]
            sy[p, lo:lo + N] = snake[b, :, 1]
    shift = np.zeros((P, P), np.float32)
    for q in range(P - B):
        shift[q, q + B] = 1.0
    return {"cx": cx, "cy": cy, "sx": sx, "sy": sy, "shift": shift}


_CACHED = {}


def _get_nc():
    if "nc" not in _CACHED:
        _CACHED["nc"] = build_bass()
    return _CACHED["nc"]


def host_finish(out_map, snake, contour):
    """Recompute the final CxC block in float64 log space from exported
    boundaries (the corner can sit beyond f32 range below the block peak).
    snake, contour: [B, N, 2] for this core. Returns R[B]."""
    LN2 = np.log(2.0)
    # Boundary cells near the f32 denormal floor flush to 0 on some runs
    # (1-ulp-level nondeterminism); log(0) would make the corner path
    # impossible and the loss inf.  Clamp at the smallest denormal: that is
    # exactly where those cells sit on runs where they survive.
    DENORM = 2.0 ** -149
    bcol = np.maximum(out_map["bcol"].astype(np.float64), DENORM)
    erow = np.maximum(out_map["erow"].astype(np.float64), DENORM)
    lh = out_map["lh"].astype(np.float64)       # [P, NHIST]
    i0 = N - C
    res = np.empty(B)
    for b in range(B):
        p15 = (K - 1) * B + b
        p14 = (K - 2) * B + b
        with np.errstate(divide="ignore"):
            # R[i0-1, j], j = i0-1 .. N-1 (erow col 0 is j = i0-1)
            sc15 = lh[p15, (EROW_STEP - TAIL0) // Q]
            Rrow = -(np.log(erow[p15]) + LN2 * sc15)
            # R[i, i0-1], i = i0 .. N-1: slot-14 last col at step i + 2(K-2)
            tt = i0 + np.arange(C) + 2 * (K - 2)
            sc = lh[p14, (tt - TAIL0) // Q]
            Rcol = -(np.log(bcol[p14, tt - TAIL0]) + LN2 * sc)
        D = ((snake[b, i0:, None, :].astype(np.float64)
              - contour[b, None, i0:, :].astype(np.float64)) ** 2).sum(-1)
        Rm = np.full((C + 1, C + 1), np.inf)
        Rm[0, :] = Rrow
        Rm[1:, 0] = Rcol
        for ii in range(1, C + 1):
            dvals = D[ii - 1]
            rowm1 = Rm[ii - 1]
            rowc = Rm[ii]
            for jj in range(1, C + 1):
                v0, v1, v2 = rowm1[jj - 1], rowm1[jj], rowc[jj - 1]
                mn = min(v0, v1, v2)
                if mn == np.inf:
                    continue
                rowc[jj] = dvals[jj - 1] + mn - np.log(
                    np.exp(mn - v0) + np.exp(mn - v1) + np.exp(mn - v2)
                )
        res[b] = Rm[C, C]
    return res


def run(snake, contour, trace=False):
    """Returns (loss, results_obj)."""
    snake = np.asarray(snake, np.float32)
    contour = np.asarray(contour, np.float32)
    nbatch = snake.shape[0]
    assert nbatch == NCORES * B, (snake.shape, contour.shape)
    in_maps = [
        prep_core_inputs(
            snake[c * B:(c + 1) * B], contour[c * B:(c + 1) * B]
        )
        for c in range(NCORES)
    ]
    nc = _get_nc()
    res = bass_utils.run_bass_kernel_spmd(
        nc, in_maps, core_ids=list(range(NCORES)), trace=trace
    )
    rs = []
    for c in range(NCORES):
        rs.append(host_finish(
            res.results[c],
            snake[c * B:(c + 1) * B], contour[c * B:(c + 1) * B],
        ))
    loss = np.mean(np.concatenate(rs), dtype=np.float64)
    return np.float32(loss), res


def kernel(snake, contour):
    loss, _ = run(snake, contour, trace=False)
    return np.array(loss, dtype=np.float32)
